# revision 1
# baseline (speedup 1.0000x reference)
"""Trainium2 Bass kernel for im2col Conv2d dot-product:
out[b, n] = <enc_x[b, n, :], w_flat> + bias.

Data-parallel over batch: 8 batches per NeuronCore x 8 cores.
Per core: x is [401408, 49] -> out [401408] fp32.

v5: dense phase-packed TensorE matmul.

DMA on TRN2 runs at full rate (~427 GB/s, 27 GB/s x 16 engines)
ONLY for 128-partition tiles (measured: 98 partitions -> ~260 GB/s,
112 -> ~263, 120 -> ~199). So the moving operand must fill all 128
partitions with real data, but windows are 49 long: instead of
2-windows-per-column (98 rows + 30 pad), pack the im2col stream
DENSELY: column c of "phase" phi holds flat element 128*phi + c of a
6272-element group (= 128 windows x 49 = lcm(49,128) structure).
The identity 128*phi + c = 49*m + k uniquely assigns every (phi, c)
to window-row m, element k, so 49 banded stationaries
S_phi[c, m] = w[128*phi + c - 49*m] (in [0,49)) make the 49
phase-matmuls accumulate exact per-window dot products into
PSUM[m, g] with zero padding and zero redundancy.

Stream order puts window m*3136+g at stream slot 128*g+m, so
PSUM[m, g-chunk] lands in natural window order: out DMA is
contiguous per partition.  Each 448-column block closes after its
49 matmuls -> ScalarE bias-add -> out DMA, fully pipelined (no
end-of-kernel PSUM flush tail).

Per core: DMA in 39.3 MB bf16 (~95-105 us at full rate), TensorE
153664 moving columns ~ 64 us @2.4 GHz, ScalarE 3.6 us, DVE idle.
"""

from contextlib import ExitStack

import numpy as np
import ml_dtypes

import concourse.bass as bass
import concourse.tile as tile
from concourse import mybir

B = 64
WINDOWS = 50176
K = 49
NCORES = 8
BPC = B // NCORES            # batches per core
NWIN = BPC * WINDOWS         # 401408 windows per core
ROWS = 128                   # window-row grid: window = m*J + g
J = NWIN // ROWS             # 3136
NPHI = K                     # 49 phases
GW = 448                     # g-columns per full block
# last full block split in two: shorter tail after the final x-DMA
BLOCKS = [448] * 6 + [112] * 4           # g-cols per block, sum = J
BATCHES = [(0, 3), (3, 3), (6, 4)]       # out-DMA batching (start, nblocks)
NBLK = len(BLOCKS)

FP32 = mybir.dt.float32
BF16 = mybir.dt.bfloat16
BF16_NP = ml_dtypes.bfloat16

_NC = None


def _build_nc():
    nc = bass.Bass(trn_type="TRN2", debug=False, num_devices=NCORES)

    # x: concatenated per-block [ROWS, NPHI*gw_b] slabs (column phi*gw_b+gw)
    x = nc.dram_tensor("x", [ROWS * NPHI * J], BF16, kind="ExternalInput").ap()
    # compact stationary band: per phase, the <=4 nonzero columns
    s = nc.dram_tensor("s", [ROWS, NPHI * 4], BF16, kind="ExternalInput").ap()
    b = nc.dram_tensor("b", [1], FP32, kind="ExternalInput").ap()
    out = nc.dram_tensor("out", [NWIN], FP32, kind="ExternalOutput").ap()

    with tile.TileContext(nc) as tc, ExitStack() as ctx:
        consts = ctx.enter_context(tc.tile_pool(name="consts", bufs=1))
        xpool = ctx.enter_context(tc.tile_pool(name="x", bufs=4))
        opool = ctx.enter_context(tc.tile_pool(name="o", bufs=2))
        ppool = ctx.enter_context(tc.tile_pool(name="psum", bufs=4, space="PSUM"))

        # stat is mostly zeros (<=4 nonzero cols per phase): DMA only the
        # 100 KB compact band and scatter with the otherwise-idle DVE, so
        # the full 1.6 MB never contends with the x stream (it previously
        # landed ~30 us in, starving the PE's start).
        sc = consts.tile([ROWS, NPHI * 4], BF16)
        nc.gpsimd.dma_start(out=sc[:], in_=s)
        bb = consts.tile([ROWS, 1], FP32)
        nc.gpsimd.dma_start(
            out=bb[:],
            in_=bass.AP(tensor=b.tensor, offset=b.offset, ap=[[0, ROWS]] + list(b.ap)),
        )
        stat = consts.tile([ROWS, NPHI * ROWS], BF16)
        nc.vector.memset(stat[:], 0.0)
        for phi in range(NPHI):
            m_lo = (128 * phi) // K
            width = min(4, ROWS - m_lo)
            nc.vector.tensor_copy(
                out=stat[:, phi * ROWS + m_lo : phi * ROWS + m_lo + width],
                in_=sc[:, phi * 4 : phi * 4 + width],
            )

        blk_g0 = [sum(BLOCKS[:i]) for i in range(NBLK)]   # g-offset per block
        xoff = 0
        for bstart, bn in BATCHES:
            bcols = sum(BLOCKS[bstart : bstart + bn])
            obuf = opool.tile([ROWS, bcols], FP32, tag="ot", name=f"ot{bstart}")
            ocol = 0
            for blk in range(bstart, bstart + bn):
                gw = BLOCKS[blk]
                xt = xpool.tile([ROWS, NPHI * gw], BF16, tag="xt", name=f"xt{blk}")
                src = bass.AP(
                    tensor=x.tensor,
                    offset=x.offset + xoff,
                    ap=[[NPHI * gw, ROWS], [1, NPHI * gw]],
                )
                xoff += ROWS * NPHI * gw
                nc.sync.dma_start(out=xt[:], in_=src)

                # full PSUM bank (512 fp32) for alignment; use gw cols
                acc = ppool.tile([ROWS, 512], FP32, tag="acc", name=f"acc{blk}")
                for phi in range(NPHI):
                    nc.tensor.matmul(
                        acc[:, 0:gw],
                        stat[:, phi * ROWS : (phi + 1) * ROWS],
                        xt[:, phi * gw : (phi + 1) * gw],
                        start=(phi == 0),
                        stop=(phi == NPHI - 1),
                    )

                nc.scalar.activation(
                    out=obuf[:, ocol : ocol + gw],
                    in_=acc[:, 0:gw],
                    func=mybir.ActivationFunctionType.Identity,
                    bias=bb[:, 0:1],
                    scale=1.0,
                )
                ocol += gw

            dst = bass.AP(
                tensor=out.tensor,
                offset=out.offset + blk_g0[bstart],
                ap=[[J, ROWS], [1, bcols]],
            )
            # final batch rides the (by-then idle) HWDGE sync queue
            oeng = nc.sync if bstart + bn == NBLK else nc.gpsimd
            oeng.dma_start(out=dst, in_=obuf[:])

    return nc


def _split_ctrl_waits(nc, max_waits=1):
    """Work around a walrus codegen limit on this build: instructions accept
    only one sync-wait command. Hoist extra waits onto dedicated no-op
    instructions inserted just before, preserving per-engine order."""
    from concourse import mybir

    for f in nc.m.functions:
        for blk in f.blocks:
            insts = blk.instructions
            i = 0
            while i < len(insts):
                ins = insts[i]
                if (
                    ins.sync_info is not None
                    and len(ins.sync_info.on_wait) > max_waits
                ):
                    waits = list(ins.sync_info.on_wait)
                    keep, extra = waits[:max_waits], waits[max_waits:]
                    ins.sync_info.on_wait = keep
                    for j, wchunk in enumerate(extra):
                        nop = mybir.InstNoOp(
                            name=f"{ins.name}-wsplit{j}",
                            sync_info=mybir.SyncInfo(on_wait=[wchunk], on_update=[]),
                            bass_nofuse=True,
                            engine=ins.engine,
                        )
                        nc.register_instruction(nop, overwrite=True)
                        insts.insert(i, nop)
                        i += 1
                i += 1


def _get_nc():
    global _NC
    if _NC is None:
        _NC = _build_nc()
        _split_ctrl_waits(_NC)
    return _NC


# z = 128*phi + c = 49*m + k for z in [0, 6272)
_Z = np.arange(ROWS * K)
_MZ = _Z // K
_KZ = _Z % K


def _pack_inputs(enc_x, weight, bias):
    """Host-side repack: dense phase-packed bf16 layout + banded stationaries."""
    # xr[m, g, k] = enc_x_core[m*J + g, k]; phase view:
    # X4[phi, c, g] = xr[mz, g, kz] at z = 128*phi + c
    xr = np.asarray(enc_x, dtype=np.float32).reshape(NCORES, ROWS, J, K)
    xb = xr.astype(BF16_NP)
    xT = np.empty((NCORES, ROWS * NPHI * J), dtype=BF16_NP)
    for cix in range(NCORES):
        g1 = xb[cix][_MZ, :, _KZ]                      # [6272, J]
        x4 = g1.reshape(NPHI, ROWS, J)                 # [phi, c, g]
        slabs, g0 = [], 0
        for gw in BLOCKS:
            slabs.append(
                np.ascontiguousarray(
                    x4[:, :, g0 : g0 + gw].transpose(1, 0, 2)
                ).reshape(-1)
            )
            g0 += gw
        xT[cix] = np.concatenate(slabs)

    wb = np.asarray(weight, dtype=np.float32).reshape(K).astype(BF16_NP)
    # compact band: col phi*4 + (m - m_lo(phi)) holds stat col phi*128 + m
    stat = np.zeros((ROWS, NPHI * 4), dtype=BF16_NP)
    for phi in range(NPHI):
        z = 128 * phi + np.arange(ROWS)                # z for each c
        m = z // K
        k = z % K
        m_lo = (128 * phi) // K
        stat[np.arange(ROWS), phi * 4 + (m - m_lo)] = wb[k]

    bf = np.ascontiguousarray(np.asarray(bias), dtype=np.float32).reshape(1)
    return xT, stat, bf


def run(enc_x, weight, bias, trace=False, **spmd_kwargs):
    """Run on 8 NeuronCores; returns (out [B, WINDOWS] fp32, BassKernelResults)."""
    from concourse.bass_utils import run_bass_kernel_spmd

    nc = _get_nc()
    xT, stat, bf = _pack_inputs(enc_x, weight, bias)
    in_maps = [{"x": xT[i], "s": stat, "b": bf} for i in range(NCORES)]
    try:
        res = run_bass_kernel_spmd(
            nc, in_maps, list(range(NCORES)), trace=trace, **spmd_kwargs
        )
    except Exception:
        # rare transient NRT_EXEC_UNIT_UNRECOVERABLE: retry once
        res = run_bass_kernel_spmd(
            nc, in_maps, list(range(NCORES)), trace=trace, **spmd_kwargs
        )
    out = np.stack([res.results[i]["out"] for i in range(NCORES)], axis=0)
    return out.reshape(B, WINDOWS), res


def kernel(enc_x, weight, bias, windows_nb=None):
    out, _ = run(enc_x, weight, bias)
    return out



# revision 5
# speedup vs baseline: 1.7235x; 1.7235x over previous
"""Trainium2 Bass kernel for im2col Conv2d dot-product:
out[b, n] = <enc_x[b, n, :], w_flat> + bias.

Data-parallel over batch: 8 batches per NeuronCore x 8 cores.
Per core: x is [401408, 49] -> out [401408] fp32.

v5: dense phase-packed TensorE matmul.

DMA on TRN2 runs at full rate (~427 GB/s, 27 GB/s x 16 engines)
ONLY for 128-partition tiles (measured: 98 partitions -> ~260 GB/s,
112 -> ~263, 120 -> ~199). So the moving operand must fill all 128
partitions with real data, but windows are 49 long: instead of
2-windows-per-column (98 rows + 30 pad), pack the im2col stream
DENSELY: column c of "phase" phi holds flat element 128*phi + c of a
6272-element group (= 128 windows x 49 = lcm(49,128) structure).
The identity 128*phi + c = 49*m + k uniquely assigns every (phi, c)
to window-row m, element k, so 49 banded stationaries
S_phi[c, m] = w[128*phi + c - 49*m] (in [0,49)) make the 49
phase-matmuls accumulate exact per-window dot products into
PSUM[m, g] with zero padding and zero redundancy.

Stream order puts window m*3136+g at stream slot 128*g+m, so
PSUM[m, g-chunk] lands in natural window order: out DMA is
contiguous per partition.  Each 448-column block closes after its
49 matmuls -> ScalarE bias-add -> out DMA, fully pipelined (no
end-of-kernel PSUM flush tail).

Per core: DMA in 39.3 MB bf16 (~95-105 us at full rate), TensorE
153664 moving columns ~ 64 us @2.4 GHz, ScalarE 3.6 us, DVE idle.
"""

from contextlib import ExitStack

import numpy as np
import ml_dtypes

import concourse.bass as bass
import concourse.tile as tile
from concourse import mybir

B = 64
WINDOWS = 50176
K = 49
NCORES = 8
BPC = B // NCORES            # batches per core
NWIN = BPC * WINDOWS         # 401408 windows per core
ROWS = 128                   # window-row grid: window = m*J + g
J = NWIN // ROWS             # 3136
NPHI = K                     # 49 phases
GW = 448                     # g-columns per full block
# last full block split in two: shorter tail after the final x-DMA
BLOCKS = [448] * 6 + [112] * 4           # g-cols per block, sum = J
BATCHES = [(0, 3), (3, 3), (6, 4)]       # out-DMA batching (start, nblocks)
NBLK = len(BLOCKS)

FP32 = mybir.dt.float32
BF16 = mybir.dt.bfloat16
F8E3 = mybir.dt.float8e3
BF16_NP = ml_dtypes.bfloat16
F8E3_NP = ml_dtypes.float8_e3m4

_NC = None


def _build_nc():
    nc = bass.Bass(trn_type="TRN2", debug=False, num_devices=NCORES)

    # x: concatenated per-block [ROWS, NPHI*gw_b] slabs (column phi*gw_b+gw)
    # fp8 e3m4: halves the HBM stream vs bf16; measured rel err 1.5e-2 vs
    # the 2e-2 gate (weights stay bf16 -- mixed-dtype matmul is exact on HW)
    x = nc.dram_tensor("x", [ROWS * NPHI * J], F8E3, kind="ExternalInput").ap()
    # compact stationary band: per phase, the <=4 nonzero columns
    s = nc.dram_tensor("s", [ROWS, NPHI * 4], BF16, kind="ExternalInput").ap()
    b = nc.dram_tensor("b", [1], FP32, kind="ExternalInput").ap()
    out = nc.dram_tensor("out", [NWIN], FP32, kind="ExternalOutput").ap()

    with tile.TileContext(nc) as tc, ExitStack() as ctx:
        consts = ctx.enter_context(tc.tile_pool(name="consts", bufs=1))
        xpool = ctx.enter_context(tc.tile_pool(name="x", bufs=4))
        opool = ctx.enter_context(tc.tile_pool(name="o", bufs=2))
        ppool = ctx.enter_context(tc.tile_pool(name="psum", bufs=4, space="PSUM"))

        # stat is mostly zeros (<=4 nonzero cols per phase): DMA only the
        # 100 KB compact band and scatter with the otherwise-idle DVE, so
        # the full 1.6 MB never contends with the x stream (it previously
        # landed ~30 us in, starving the PE's start).
        sc = consts.tile([ROWS, NPHI * 4], BF16)
        nc.gpsimd.dma_start(out=sc[:], in_=s)
        bb = consts.tile([ROWS, 1], FP32)
        nc.gpsimd.dma_start(
            out=bb[:],
            in_=bass.AP(tensor=b.tensor, offset=b.offset, ap=[[0, ROWS]] + list(b.ap)),
        )
        stat = consts.tile([ROWS, NPHI * ROWS], BF16)
        nc.vector.memset(stat[:], 0.0)
        for phi in range(NPHI):
            m_lo = (128 * phi) // K
            width = min(4, ROWS - m_lo)
            nc.vector.tensor_copy(
                out=stat[:, phi * ROWS + m_lo : phi * ROWS + m_lo + width],
                in_=sc[:, phi * 4 : phi * 4 + width],
            )

        blk_g0 = [sum(BLOCKS[:i]) for i in range(NBLK)]   # g-offset per block
        xoff = 0
        for bstart, bn in BATCHES:
            bcols = sum(BLOCKS[bstart : bstart + bn])
            obuf = opool.tile([ROWS, bcols], FP32, tag="ot", name=f"ot{bstart}")
            ocol = 0
            for blk in range(bstart, bstart + bn):
                gw = BLOCKS[blk]
                xt = xpool.tile([ROWS, NPHI * gw], F8E3, tag="xt", name=f"xt{blk}")
                src = bass.AP(
                    tensor=x.tensor,
                    offset=x.offset + xoff,
                    ap=[[NPHI * gw, ROWS], [1, NPHI * gw]],
                )
                xoff += ROWS * NPHI * gw
                nc.sync.dma_start(out=xt[:], in_=src)

                # full PSUM bank (512 fp32) for alignment; use gw cols
                acc = ppool.tile([ROWS, 512], FP32, tag="acc", name=f"acc{blk}")
                for phi in range(NPHI):
                    nc.tensor.matmul(
                        acc[:, 0:gw],
                        stat[:, phi * ROWS : (phi + 1) * ROWS],
                        xt[:, phi * gw : (phi + 1) * gw],
                        start=(phi == 0),
                        stop=(phi == NPHI - 1),
                    )

                nc.scalar.activation(
                    out=obuf[:, ocol : ocol + gw],
                    in_=acc[:, 0:gw],
                    func=mybir.ActivationFunctionType.Identity,
                    bias=bb[:, 0:1],
                    scale=1.0,
                )
                ocol += gw

            dst = bass.AP(
                tensor=out.tensor,
                offset=out.offset + blk_g0[bstart],
                ap=[[J, ROWS], [1, bcols]],
            )
            # final batch rides the (by-then idle) HWDGE sync queue
            oeng = nc.sync if bstart + bn == NBLK else nc.gpsimd
            oeng.dma_start(out=dst, in_=obuf[:])

    return nc


def _split_ctrl_waits(nc, max_waits=1):
    """Work around a walrus codegen limit on this build: instructions accept
    only one sync-wait command. Hoist extra waits onto dedicated no-op
    instructions inserted just before, preserving per-engine order."""
    from concourse import mybir

    for f in nc.m.functions:
        for blk in f.blocks:
            insts = blk.instructions
            i = 0
            while i < len(insts):
                ins = insts[i]
                if (
                    ins.sync_info is not None
                    and len(ins.sync_info.on_wait) > max_waits
                ):
                    waits = list(ins.sync_info.on_wait)
                    keep, extra = waits[:max_waits], waits[max_waits:]
                    ins.sync_info.on_wait = keep
                    for j, wchunk in enumerate(extra):
                        nop = mybir.InstNoOp(
                            name=f"{ins.name}-wsplit{j}",
                            sync_info=mybir.SyncInfo(on_wait=[wchunk], on_update=[]),
                            bass_nofuse=True,
                            engine=ins.engine,
                        )
                        nc.register_instruction(nop, overwrite=True)
                        insts.insert(i, nop)
                        i += 1
                i += 1


def _get_nc():
    global _NC
    if _NC is None:
        _NC = _build_nc()
        _split_ctrl_waits(_NC)
    return _NC


# z = 128*phi + c = 49*m + k for z in [0, 6272)
_Z = np.arange(ROWS * K)
_MZ = _Z // K
_KZ = _Z % K


def _pack_inputs(enc_x, weight, bias):
    """Host-side repack: dense phase-packed bf16 layout + banded stationaries."""
    # xr[m, g, k] = enc_x_core[m*J + g, k]; phase view:
    # X4[phi, c, g] = xr[mz, g, kz] at z = 128*phi + c
    xr = np.asarray(enc_x, dtype=np.float32).reshape(NCORES, ROWS, J, K)
    xb = xr.astype(F8E3_NP)
    xT = np.empty((NCORES, ROWS * NPHI * J), dtype=F8E3_NP)
    for cix in range(NCORES):
        g1 = xb[cix][_MZ, :, _KZ]                      # [6272, J]
        x4 = g1.reshape(NPHI, ROWS, J)                 # [phi, c, g]
        slabs, g0 = [], 0
        for gw in BLOCKS:
            slabs.append(
                np.ascontiguousarray(
                    x4[:, :, g0 : g0 + gw].transpose(1, 0, 2)
                ).reshape(-1)
            )
            g0 += gw
        xT[cix] = np.concatenate(slabs)

    wb = np.asarray(weight, dtype=np.float32).reshape(K).astype(BF16_NP)
    # compact band: col phi*4 + (m - m_lo(phi)) holds stat col phi*128 + m
    stat = np.zeros((ROWS, NPHI * 4), dtype=BF16_NP)
    for phi in range(NPHI):
        z = 128 * phi + np.arange(ROWS)                # z for each c
        m = z // K
        k = z % K
        m_lo = (128 * phi) // K
        stat[np.arange(ROWS), phi * 4 + (m - m_lo)] = wb[k]

    bf = np.ascontiguousarray(np.asarray(bias), dtype=np.float32).reshape(1)
    return xT, stat, bf


def run(enc_x, weight, bias, trace=False, **spmd_kwargs):
    """Run on 8 NeuronCores; returns (out [B, WINDOWS] fp32, BassKernelResults)."""
    from concourse.bass_utils import run_bass_kernel_spmd

    nc = _get_nc()
    xT, stat, bf = _pack_inputs(enc_x, weight, bias)
    in_maps = [{"x": xT[i], "s": stat, "b": bf} for i in range(NCORES)]
    try:
        res = run_bass_kernel_spmd(
            nc, in_maps, list(range(NCORES)), trace=trace, **spmd_kwargs
        )
    except Exception:
        # rare transient NRT_EXEC_UNIT_UNRECOVERABLE: retry once
        res = run_bass_kernel_spmd(
            nc, in_maps, list(range(NCORES)), trace=trace, **spmd_kwargs
        )
    out = np.stack([res.results[i]["out"] for i in range(NCORES)], axis=0)
    return out.reshape(B, WINDOWS), res


def kernel(enc_x, weight, bias, windows_nb=None):
    out, _ = run(enc_x, weight, bias)
    return out



# revision 8
# speedup vs baseline: 1.8870x; 1.0948x over previous
"""Trainium2 Bass kernel for im2col Conv2d dot-product:
out[b, n] = <enc_x[b, n, :], w_flat> + bias.

Data-parallel over batch: 8 batches per NeuronCore x 8 cores.
Per core: x is [401408, 49] -> out [401408] fp32.

v7: fp8 e3m4 x-stream + 2-way PE column tiling.

DMA floor: 19.66 MB/core at ~430 GB/s ~= 46 us. At 1 moving col/cycle
the PE needs 64 us (153664 cols @ 2.4 GHz) and becomes the bottleneck,
so the 128x128 array is split into two independent 128x64 column tiles
(T0 -> PSUM partitions 0-63, T1 -> 64-127) that stream two moving
operands concurrently -- PE wall time ~33 us, back under the DMA floor.

Packing: windows w = p*J + g (p in [0,128), J=3136). Column tile j owns
window rows p in [64j, 64j+64). Within a half, the 3136 elements per g
(64 windows x 49) pack densely as u = 128*phi + c: 24 full 128-row
phases + one 64-row tail phase. The banded stationaries
S_phi[c, mloc] = w[(128 phi + c) % 49] at mloc = (128 phi + c)//49 are
IDENTICAL for both halves (only the moving data differs). Tail-phase
matmuls stay in 128x64 mode (no PE mode switch): contraction runs over
all 128 rows with the unused 64 stationary rows zeroed, the moving
tail tile carrying half0's tail on partitions 0-63 and half1's on
64-127 (one clean DMA, and the "garbage" rows are real finite fp8).

x in fp8 e3m4 (rel err 1.5e-2 vs the 2e-2 gate, measured); stationary
in bf16 -- mixed-dtype matmul verified exact on HW incl. subnormals.
"""

from contextlib import ExitStack

import numpy as np
import ml_dtypes

import concourse.bass as bass
import concourse.tile as tile
from concourse import mybir

B = 64
WINDOWS = 50176
K = 49
NCORES = 8
BPC = B // NCORES            # batches per core
NWIN = BPC * WINDOWS         # 401408 windows per core
ROWS = 128                   # window-row grid: window = p*J + g
J = NWIN // ROWS             # 3136
H = 64                       # windows per column-tile half
UPH = H * K                  # 3136 u-slots per half per g
NFULL = UPH // ROWS          # 24 full phases
TAILC = UPH - NFULL * ROWS   # 64 c-rows in the tail phase
NPHI = NFULL + 1             # 25
# uniform cadence (PE idle per block stays < the 3.4us HAM window) with a
# shorter final pair to cut the end-of-stream drain tail
BLOCKS = [448, 448, 448, 448, 448, 448, 224, 112, 112]
BATCHES = [(0, 3), (3, 3), (6, 3)]           # out-DMA batching (start, nblocks)
NBLK = len(BLOCKS)
NWARM = 30                   # warm-up matmuls (HAM un-throttle during ramp)
FILLERS = {448: 5, 224: 2}   # inter-block filler matmuls (N=448)
NOFILL_AFTER = 4             # keep the drain path free of fillers

FP32 = mybir.dt.float32
BF16 = mybir.dt.bfloat16
F8E3 = mybir.dt.float8e3
BF16_NP = ml_dtypes.bfloat16
F8E3_NP = ml_dtypes.float8_e3m4
FP16 = mybir.dt.float16

# bytes per g-col of one block's stream: 2 halves x 24 phases + 1 tail row-set
COLBYTES = ROWS * (2 * NFULL + 1)

_NC = None


def _build_nc():
    nc = bass.Bass(trn_type="TRN2", debug=False, num_devices=NCORES)

    # x: per-block [half0 main | half1 main | joint tail] slabs, c-major
    x = nc.dram_tensor("x", [COLBYTES * J], F8E3, kind="ExternalInput").ap()
    # full stationaries (zeros included): 25 phase bands [128, 64] then the
    # two tail stationaries [128, 64] each. Only ~0.44 MB -- a direct DMA
    # beats the memset+scatter dependency chain that stalled the first MM.
    s = nc.dram_tensor("s", [ROWS, (NFULL + 2) * 4 + 1], BF16, kind="ExternalInput").ap()
    b = nc.dram_tensor("b", [1], FP32, kind="ExternalInput").ap()
    # fp16 output stream: halves the writeback bytes; adds ~2^-11 rel err
    out = nc.dram_tensor("out", [NWIN], FP16, kind="ExternalOutput").ap()

    with tile.TileContext(nc) as tc, ExitStack() as ctx:
        consts = ctx.enter_context(tc.tile_pool(name="consts", bufs=1))
        xpool = ctx.enter_context(tc.tile_pool(name="x", bufs=6))
        opool = ctx.enter_context(tc.tile_pool(name="o", bufs=2))
        ppool = ctx.enter_context(tc.tile_pool(name="psum", bufs=5, space="PSUM"))
        wpool = ctx.enter_context(tc.tile_pool(name="warm", bufs=1, space="PSUM"))

        # compact band (27 KB) + DVE memset/scatter instead of the 0.44 MB
        # full-stationary DMA: frees ~1 us of HBM stream; the scatter runs
        # during the preamble (PE start is covered by the junk warm-up)
        sc = consts.tile([ROWS, (NFULL + 2) * 4 + 1], BF16)
        nc.scalar.dma_start(out=sc[:], in_=s)
        stat = consts.tile([ROWS, (NFULL + 2) * H], BF16)
        nc.vector.memset(stat[:], 0.0)
        for phi in range(NFULL):
            m_lo = (ROWS * phi) // K
            width = (ROWS * phi + ROWS - 1) // K - m_lo + 1
            nc.vector.tensor_copy(
                out=stat[:, phi * H + m_lo : phi * H + m_lo + width],
                in_=sc[:, phi * 4 : phi * 4 + width],
            )
        t_lo = (ROWS * NFULL) // K
        t_w = (UPH - 1) // K - t_lo + 1
        nc.vector.tensor_copy(
            out=stat[0:TAILC, NFULL * H + t_lo : NFULL * H + t_lo + t_w],
            in_=sc[0:TAILC, NFULL * 4 : NFULL * 4 + t_w],
        )
        nc.vector.tensor_copy(
            out=stat[ROWS - TAILC :, (NFULL + 1) * H + t_lo : (NFULL + 1) * H + t_lo + t_w],
            in_=sc[ROWS - TAILC :, (NFULL + 1) * 4 : (NFULL + 1) * 4 + t_w],
        )
        bb = sc[:, (NFULL + 2) * 4 : (NFULL + 2) * 4 + 1]
        # keep one early SWDGE op so the Q7 boot is pre-paid for the outs
        bq = consts.tile([ROWS, 1], FP32)
        nc.gpsimd.dma_start(
            out=bq[:],
            in_=bass.AP(tensor=b.tensor, offset=b.offset, ap=[[0, ROWS]] + list(b.ap)),
        )
        statt = stat[:, NFULL * H : (NFULL + 2) * H]

        # warm-up / filler source: zeroed by a fast boot-time DVE memset
        # (not a DMA), so these matmuls run during the runtime preamble,
        # before any data lands, keeping the PE at full clock throughout
        junk = consts.tile([ROWS, 512], BF16, tag="junk", name="junk")
        nc.vector.memset(junk[:], 0.0)
        warm = wpool.tile([ROWS, 512], FP32, tag="warm", name="warm")

        def pe_filler(n, cols):
            for _ in range(n):
                nc.tensor.matmul(
                    warm[0:H, 0:cols],
                    junk[:, 0:H],
                    junk[:, 0:cols],
                    start=True,
                    stop=True,
                    tile_position=(0, 0),
                )

        pe_filler(NWARM, 512)

        blk_g0 = [sum(BLOCKS[:i]) for i in range(NBLK)]   # g-offset per block
        xoff = 0
        for bstart, bn in BATCHES:
            bcols = sum(BLOCKS[bstart : bstart + bn])
            obuf = opool.tile([ROWS, bcols], FP16, tag="ot", name=f"ot{bstart}")
            ocol = 0
            for blk in range(bstart, bstart + bn):
                gw = BLOCKS[blk]
                xts = []
                for j in range(2):
                    xt = xpool.tile(
                        [ROWS, NFULL * gw], F8E3, tag=f"xt{j}", name=f"xt{j}_{blk}"
                    )
                    src = bass.AP(
                        tensor=x.tensor,
                        offset=x.offset + xoff,
                        ap=[[NFULL * gw, ROWS], [1, NFULL * gw]],
                    )
                    xoff += ROWS * NFULL * gw
                    # split the halves across the two HWDGE rings to overlap
                    # per-DMA fixed costs
                    (nc.sync if j == 0 else nc.scalar).dma_start(out=xt[:], in_=src)
                    xts.append(xt)
                xtt = xpool.tile([ROWS, gw], F8E3, tag="xtt", name=f"xtt{blk}")
                src = bass.AP(
                    tensor=x.tensor,
                    offset=x.offset + xoff,
                    ap=[[gw, ROWS], [1, gw]],
                )
                xoff += ROWS * gw
                nc.sync.dma_start(out=xtt[:], in_=src)

                # full PSUM bank (512 fp32) for alignment; use gw cols
                acc = ppool.tile([ROWS, 512], FP32, tag="acc", name=f"acc{blk}")
                for phi in range(NFULL):
                    for j in range(2):
                        nc.tensor.matmul(
                            acc[j * H : (j + 1) * H, 0:gw],
                            stat[:, phi * H : (phi + 1) * H],
                            xts[j][:, phi * gw : (phi + 1) * gw],
                            start=(phi == 0),
                            stop=False,
                            tile_position=(0, j * H),
                        )
                for j in range(2):
                    nc.tensor.matmul(
                        acc[j * H : (j + 1) * H, 0:gw],
                        statt[:, j * H : (j + 1) * H],
                        xtt[:, 0:gw],
                        start=False,
                        stop=True,
                        tile_position=(0, j * H),
                    )

                nc.scalar.activation(
                    out=obuf[:, ocol : ocol + gw],
                    in_=acc[:, 0:gw],
                    func=mybir.ActivationFunctionType.Identity,
                    bias=bb,
                    scale=1.0,
                )
                ocol += gw

                if blk < NOFILL_AFTER:
                    pe_filler(FILLERS[gw], 448)


            dst = bass.AP(
                tensor=out.tensor,
                offset=out.offset + blk_g0[bstart],
                ap=[[J, ROWS], [1, bcols]],
            )
            # final batch rides the (by-then idle) HWDGE sync queue
            oeng = nc.sync if bstart + bn == NBLK else nc.gpsimd
            oeng.dma_start(out=dst, in_=obuf[:])

    return nc


def _split_ctrl_waits(nc, max_waits=1):
    """Work around a walrus codegen limit on this build: instructions accept
    only one sync-wait command. Hoist extra waits onto dedicated no-op
    instructions inserted just before, preserving per-engine order."""
    from concourse import mybir

    for f in nc.m.functions:
        for blk in f.blocks:
            insts = blk.instructions
            i = 0
            while i < len(insts):
                ins = insts[i]
                if (
                    ins.sync_info is not None
                    and len(ins.sync_info.on_wait) > max_waits
                ):
                    waits = list(ins.sync_info.on_wait)
                    keep, extra = waits[:max_waits], waits[max_waits:]
                    ins.sync_info.on_wait = keep
                    for j, wchunk in enumerate(extra):
                        nop = mybir.InstNoOp(
                            name=f"{ins.name}-wsplit{j}",
                            sync_info=mybir.SyncInfo(on_wait=[wchunk], on_update=[]),
                            bass_nofuse=True,
                            engine=ins.engine,
                        )
                        nc.register_instruction(nop, overwrite=True)
                        insts.insert(i, nop)
                        i += 1
                i += 1


def _get_nc():
    global _NC
    if _NC is None:
        _NC = _build_nc()
        _split_ctrl_waits(_NC)
    return _NC


# u = 128*phi + c = 49*mloc + k for u in [0, 3136) (per 64-window half)
_U = np.arange(UPH)
_MU = _U // K
_KU = _U % K


def _pack_inputs(enc_x, weight, bias):
    """Host-side repack: per-half dense phase-packed fp8 layout + banded
    bf16 stationaries."""
    xr = np.asarray(enc_x, dtype=np.float32).reshape(NCORES, ROWS, J, K)
    xb = xr.astype(F8E3_NP)
    xT = np.empty((NCORES, COLBYTES * J), dtype=F8E3_NP)
    for cix in range(NCORES):
        # G[j] = [3136, J]: u-slot-major elements of half j
        G = [xb[cix][H * j + _MU, :, _KU] for j in range(2)]
        slabs, g0 = [], 0
        for gw in BLOCKS:
            for j in range(2):
                # [c, phi, gw]
                slabs.append(
                    np.ascontiguousarray(
                        G[j][: NFULL * ROWS, g0 : g0 + gw]
                        .reshape(NFULL, ROWS, gw)
                        .transpose(1, 0, 2)
                    ).reshape(-1)
                )
            tail = np.concatenate(
                [G[j][NFULL * ROWS :, g0 : g0 + gw] for j in range(2)], axis=0
            )
            slabs.append(np.ascontiguousarray(tail).reshape(-1))
            g0 += gw
        xT[cix] = np.concatenate(slabs)

    wb = np.asarray(weight, dtype=np.float32).reshape(K).astype(BF16_NP)
    # compact band: col phi*4 + (u//K - m_lo(phi)); tail bands for half0
    # (rows 0-63) / half1 (rows 64-127); bias in the last column
    stat = np.zeros((ROWS, (NFULL + 2) * 4 + 1), dtype=BF16_NP)
    for phi in range(NFULL):
        u = ROWS * phi + np.arange(ROWS)
        m_lo = (ROWS * phi) // K
        stat[np.arange(ROWS), phi * 4 + (u // K - m_lo)] = wb[u % K]
    u = ROWS * NFULL + np.arange(TAILC)
    t_lo = (ROWS * NFULL) // K
    stat[np.arange(TAILC), NFULL * 4 + (u // K - t_lo)] = wb[u % K]
    stat[ROWS - TAILC + np.arange(TAILC), (NFULL + 1) * 4 + (u // K - t_lo)] = wb[u % K]
    stat[:, (NFULL + 2) * 4] = np.float32(np.asarray(bias).reshape(-1)[0])

    bf = np.ascontiguousarray(np.asarray(bias), dtype=np.float32).reshape(1)
    return xT, stat, bf


def run(enc_x, weight, bias, trace=False, **spmd_kwargs):
    """Run on 8 NeuronCores; returns (out [B, WINDOWS] fp32, BassKernelResults)."""
    from concourse.bass_utils import run_bass_kernel_spmd

    nc = _get_nc()
    xT, stat, bf = _pack_inputs(enc_x, weight, bias)
    in_maps = [{"x": xT[i], "s": stat, "b": bf} for i in range(NCORES)]
    try:
        res = run_bass_kernel_spmd(
            nc, in_maps, list(range(NCORES)), trace=trace, **spmd_kwargs
        )
    except Exception:
        # rare transient NRT_EXEC_UNIT_UNRECOVERABLE: retry once
        res = run_bass_kernel_spmd(
            nc, in_maps, list(range(NCORES)), trace=trace, **spmd_kwargs
        )
    out = np.stack([res.results[i]["out"].astype(np.float32) for i in range(NCORES)], axis=0)
    return out.reshape(B, WINDOWS), res


def kernel(enc_x, weight, bias, windows_nb=None):
    out, _ = run(enc_x, weight, bias)
    return out
